# revision 39
# baseline (speedup 1.0000x reference)
"""Complex-valued attention kernel for Trainium2, SPMD over 8 NeuronCores.

Problem (hardcoded shapes): B=4, N=2048, E=384, H=6, D=64, complex64.
  qkv = x @ w_qkv^T + b_qkv          (complex)
  q, k = complex RMSNorm over D (eps=1e-6), affine weights qn_w/kn_w
  scores = Re(q @ conj(k)^T) / sqrt(D)
  attn = softmax(scores)  (real), out = attn @ v   -> [B, N, E] complex64

Sharding: core c handles batch b=c//2, heads 3*(c%2)..3*(c%2)+2 (24 head-
batches over 8 cores, 3 each).

Design notes (HW ~201us vs 365us baseline; trace-verified on trn2):
  - PH1 (QKV+norm+pack, ~63us): bias folded into the PSUM->SBUF evacuation
    adds (DVE tensor_tensor, f32 bias broadcast on-device via gpsimd);
    q scaled by rq on ScalarE copies; k left UNSCALED -- rk is folded into
    the PH2 exp as a per-partition activation scale (softmax row scale is
    per-kv-token there, which is the partition dim of S^T).  Token->pack
    transposes via the DMA XBAR (dma_start_transpose, contiguous
    [128,3,128] dest -- a strided dest produces wrong data on HW) instead
    of PE transpose + DVE copy.  V bias applied on HOST (sum(attn)=1 makes
    it a constant output offset).
  - PH2 (attention, ~115us): per (head, 1024-q chunk) x 16 kv tiles:
    S^T = kpack.T @ qpack (one 128-contraction MM per 512 free), exp with
    scale=rk on ScalarE, PV accumulation.  Z row sums NOT on the PE:
    exp tiles are accumulated on DVE into two bf16 zacc buffers, DMA'd
    out raw; the 128-partition Z sum, 1/Z division and out^T -> [token,d]
    transpose all happen on host during unsharding.  PSUM budget:
    st(2 bufs x 2 banks) + pv(2 bufs x 2 banks) = 8 banks -> consecutive
    chunks overlap and the PE never idles long enough for HAM re-throttle
    (K=8/8 at 2.4 GHz through the whole phase).
  - ScalarE exp is the hard floor: 96 x [128,1024] x ~1.2us ~= 115us.
    PSUM (8 banks) rules out wider exp granules; DVE/GpSimd cannot exp.
  - Known walrus/HW landmines (do NOT reintroduce): tensor_tensor_reduce
    (HW exec fault), gpsimd tensor ops reading PSUM (walrus crash),
    rearrange-view reduce over [128,(h c)] (NaN on HW, fine in CoreSim),
    gpsimd tensor_scalar_mul (runs but ~2.1us per [128,128] -- 6x slower
    than DVE/ACT; regressed the kernel to 281us).  Also measured neutral-
    to-negative: k-square on gpsimd + ph1 psum bufs 8->6 (201.6us vs 196).
    The PH1->PH2 transition warm-keeper dummies fix the HAM re-throttle
    there (trace-confirmed) but the wall is ACT-gated at that point, so
    the gain is variance reduction only.
"""

import numpy as np

import concourse.bass as bass
import concourse.tile as tile
from concourse import bacc, mybir
from concourse.bass_utils import run_bass_kernel_spmd

B, N, E, H, D = 4, 2048, 384, 6, 64
EPS = 1e-6
HPC = 3            # heads per core
NT = N // 128      # 16 token tiles
KT = E // 128      # 3 contraction tiles
QC = 2             # q chunks of 1024
F32 = mybir.dt.float32
MMD = mybir.dt.bfloat16

_prog_cache = {}


def _widx(p, a, k):
    return (p * 2 + a) * KT + k


def build_program():
    nc = bacc.Bacc(
        "TRN2", target_bir_lowering=False, debug=False, num_devices=8)
    xt_r = nc.declare_dram_parameter("xt_r", [E, N], MMD, isOutput=False)
    xt_i = nc.declare_dram_parameter("xt_i", [E, N], MMD, isOutput=False)
    w_in = nc.declare_dram_parameter("w", [3, 2, E, 384], MMD, isOutput=False)
    b_in = nc.declare_dram_parameter("bias", [1, 2, 384], F32, isOutput=False)
    outT_d = nc.declare_dram_parameter("outT", [HPC, QC, 128, 1024], F32,
                                       isOutput=True)
    zac_d = nc.declare_dram_parameter("zac", [HPC, QC, 128, 2, 1024], MMD,
                                      isOutput=True)

    with tile.TileContext(nc) as tc:
        with tc.tile_pool(name="persist", bufs=1) as pp:
            bias_row = pp.tile([1, 2, 384], F32)
            nc.sync.dma_start(out=bias_row, in_=b_in[:])
            bias_sb = pp.tile([128, 2, 384], F32)
            nc.gpsimd.partition_broadcast(bias_sb, bias_row)
            w_sb = pp.tile([128, 3 * 2 * KT, 384], MMD)

            # packs: [d2, token-tile, head, token-in-tile]
            qpack = pp.tile([128, NT, HPC, 128], MMD)
            kpack = pp.tile([128, NT, HPC, 128], MMD)
            vpack = pp.tile([128, NT, 384], MMD)   # [token, kv-tile, head*128]
            rk_sb = pp.tile([128, NT, HPC], F32)   # per-kv-token k norm scale
            eps_q = pp.tile([128, 1], F32)
            eps_k = pp.tile([128, 1], F32)
            nc.vector.memset(eps_q, 64.0 * EPS)
            nc.vector.memset(eps_k, EPS)

            xt_sb = pp.tile([128, 2, KT, N], MMD)
            NXC = 8  # x dma chunks

            def _x_chunk(xc):
                sl = slice(xc * (N // NXC), (xc + 1) * (N // NXC))
                nc.sync.dma_start(
                    out=xt_sb[:, 0, :, sl],
                    in_=xt_r[:, sl].rearrange("(k q) n -> q k n", q=128),
                )
                nc.sync.dma_start(
                    out=xt_sb[:, 1, :, sl],
                    in_=xt_i[:, sl].rearrange("(k q) n -> q k n", q=128),
                )

            # x chunk 0 first (gates tile 0), then weights in 6 chunks
            # ordered by per-tile MM consumption ((k,a) inner loops) so the
            # first matmuls start ~4us in instead of waiting for all 3.5MB,
            # then the remaining x chunks.
            _x_chunk(0)
            for k in range(KT):
                for a in range(2):
                    nc.sync.dma_start(
                        out=w_sb[:, a * KT + k::2 * KT],
                        in_=w_in[:, a, k * 128:(k + 1) * 128, :].rearrange(
                            "p q c -> q p c"),
                    )
            for xc in range(1, NXC):
                _x_chunk(xc)

            # HAM pre-warm: dummy matmuls keep the PE busy through the
            # initial DMA wait so the 4096-cycle activity window fires and
            # PH1's real matmuls run at K=8/8 (2.4 GHz) from the start.
            wz = pp.tile([128, 512], MMD)
            nc.vector.memset(wz, 0.0)
            with tc.tile_pool(name="warm", bufs=1, space="PSUM") as pwm:
                warm_ps = pwm.tile([128, 512], F32)
                for _ in range(14):
                    nc.tensor.matmul(warm_ps, wz[:, 0:128], wz,
                                     start=True, stop=True)

            # ---------------- PH1: QKV + RMS norm + packing ----------------
            with (
                tc.tile_pool(name="ph1ps", bufs=8, space="PSUM") as pps,
                tc.tile_pool(name="ph1t", bufs=4) as pt1,
            ):
                for nt in range(NT):
                    psq = pps.tile([128, 384], F32, tag="ps")
                    psk = pps.tile([128, 384], F32, tag="ps")
                    psv = pps.tile([128, 384], F32, tag="ps")
                    for k in range(KT):
                        for a in range(2):
                            lhs = xt_sb[:, a, k, nt * 128:(nt + 1) * 128]
                            st = (k == 0 and a == 0)
                            sp = (k == KT - 1 and a == 1)
                            nc.tensor.matmul(psq, lhs, w_sb[:, _widx(0, a, k)],
                                             start=st, stop=sp)
                            nc.tensor.matmul(psk, lhs, w_sb[:, _widx(1, a, k)],
                                             start=st, stop=sp)
                            nc.tensor.matmul(psv, lhs, w_sb[:, _widx(2, a, k)],
                                             start=st, stop=sp)
                    # V: plain PSUM->SBUF bf16 evacuation (bias on host)
                    nc.scalar.copy(vpack[:, nt], psv)
                    # Q/K: add bias during evacuation
                    q2 = pt1.tile([128, 384], F32, tag="q2")
                    k2s = pt1.tile([128, 384], MMD, tag="k2s")
                    nc.vector.tensor_add(q2, psq, bias_sb[:, 0])
                    nc.vector.tensor_add(k2s, psk, bias_sb[:, 1])
                    # sum of squares per head: squares on gpsimd, reduce DVE
                    scr = pt1.tile([128, 384], MMD, tag="scr")
                    scrk = pt1.tile([128, 384], MMD, tag="scrk")
                    msq = pt1.tile([128, HPC], F32, tag="msq")
                    msk = pt1.tile([128, HPC], F32, tag="msk")
                    nc.scalar.square(scr, q2)
                    nc.vector.tensor_mul(scrk, k2s, k2s)
                    for hh in range(HPC):
                        blk = slice(hh * 128, (hh + 1) * 128)
                        nc.vector.reduce_sum(msq[:, hh:hh + 1], scr[:, blk],
                                             axis=mybir.AxisListType.X)
                        nc.vector.reduce_sum(msk[:, hh:hh + 1], scrk[:, blk],
                                             axis=mybir.AxisListType.X)
                    # q: rq = 1/(8 sqrt(ms+eps)) = 1/sqrt(sum_sq + 64 eps)
                    # k: rk = 1/sqrt(ms+eps)     = 1/sqrt((sum_sq)/64 + eps)
                    s8q = pt1.tile([128, HPC], F32, tag="s8q")
                    s8k = pt1.tile([128, HPC], F32, tag="s8k")
                    nc.scalar.activation(s8q, msq,
                                         mybir.ActivationFunctionType.Sqrt,
                                         bias=eps_q, scale=1.0)
                    nc.scalar.activation(s8k, msk,
                                         mybir.ActivationFunctionType.Sqrt,
                                         bias=eps_k, scale=1.0 / 64.0)
                    rq = pt1.tile([128, HPC], F32, tag="rq")
                    nc.vector.reciprocal(rq, s8q)
                    nc.vector.reciprocal(rk_sb[:, nt], s8k)
                    # scale q by rq (per-head per-token) while casting to bf16
                    q2s = pt1.tile([128, 384], MMD, tag="q2s")
                    for hh in range(HPC):
                        blk = slice(hh * 128, (hh + 1) * 128)
                        nc.scalar.activation(
                            q2s[:, blk], q2[:, blk],
                            mybir.ActivationFunctionType.Copy,
                            scale=rq[:, hh:hh + 1])
                    # token->pack transposes on the DMA XBAR (dest contiguous)
                    nc.sync.dma_start_transpose(out=qpack[:, nt], in_=q2s)
                    nc.sync.dma_start_transpose(out=kpack[:, nt], in_=k2s)

            # Transition warm-keeper: the PH1->PH2 PSUM handoff idles the PE
            # ~4-5us, long enough for a HAM MID window to re-throttle to
            # 1.2 GHz.  A dozen dummy matmuls sit in the PE's in-order queue
            # between the phases and keep the activity window alive; they
            # start as soon as the first PH1 bank drains.
            with tc.tile_pool(name="warm2", bufs=1, space="PSUM") as pw2:
                w2ps = pw2.tile([128, 512], F32)
                for _ in range(12):
                    nc.tensor.matmul(w2ps, wz[:, 0:128], wz,
                                     start=True, stop=True)

            # ---------------- PH2: attention ----------------
            with (
                tc.tile_pool(name="stp", bufs=2, space="PSUM") as pst,
                tc.tile_pool(name="pvp", bufs=2, space="PSUM") as ppv,
                tc.tile_pool(name="esp", bufs=4) as pes,
                tc.tile_pool(name="zcp", bufs=2) as pzc,
                tc.tile_pool(name="fsp", bufs=2) as pfs,
            ):
                for hh in range(HPC):
                    for qc in range(QC):
                        t0 = qc * 8
                        pv_ps = ppv.tile([128, 1024], F32, tag="pv")
                        zacc = pzc.tile([128, 2, 1024], MMD, tag="zc")
                        for kt in range(NT):
                            st_ps = pst.tile([128, 1024], F32, tag="st")
                            for hf in range(2):
                                nc.tensor.matmul(
                                    st_ps[:, hf * 512:(hf + 1) * 512],
                                    kpack[:, kt, hh],
                                    qpack[:, t0 + hf * 4:t0 + hf * 4 + 4, hh],
                                    start=True, stop=True)
                            es = pes.tile([128, 1024], MMD, tag="es")
                            nc.scalar.activation(
                                es, st_ps,
                                mybir.ActivationFunctionType.Exp,
                                scale=rk_sb[:, kt, hh:hh + 1])
                            for hf in range(2):
                                nc.tensor.matmul(
                                    pv_ps[:, hf * 512:(hf + 1) * 512],
                                    vpack[:, kt, hh * 128:(hh + 1) * 128],
                                    es[:, hf * 512:(hf + 1) * 512],
                                    start=(kt == 0), stop=(kt == NT - 1))
                            if kt < 2:
                                nc.vector.tensor_copy(zacc[:, kt], es)
                            else:
                                nc.vector.tensor_add(zacc[:, kt % 2],
                                                     zacc[:, kt % 2], es)
                        final_sb = pfs.tile([128, 1024], F32, tag="fin")
                        for hf in range(2):
                            sl = slice(hf * 512, (hf + 1) * 512)
                            nc.vector.tensor_copy(final_sb[:, sl], pv_ps[:, sl])
                            nc.sync.dma_start(out=outT_d[hh, qc, :, sl],
                                              in_=final_sb[:, sl])
                        for par in range(2):
                            nc.sync.dma_start(out=zac_d[hh, qc, :, par],
                                              in_=zacc[:, par])
    nc.compile()
    return nc


def _host_prep(x_real, x_imag, w_qkv, b_qkv, qn_w, kn_w):
    """Build the 8 per-core input maps (numpy only)."""
    qw_col = np.tile(qn_w, H)[:, None]            # [E,1] complex
    kw_col = np.tile(kn_w, H)[:, None]
    wq = w_qkv[0 * E:1 * E] * qw_col
    wk = w_qkv[1 * E:2 * E] * kw_col
    wv = w_qkv[2 * E:3 * E]
    bq = b_qkv[0 * E:1 * E] * qw_col[:, 0]
    bk = b_qkv[1 * E:2 * E] * kw_col[:, 0]

    import ml_dtypes
    bf16 = ml_dtypes.bfloat16
    in_maps = []
    for c in range(8):
        b = c // 2
        h0 = HPC * (c % 2)
        # weight tiles: w[pack, plane, e, col] with col = hh*128 + 2d (+1)
        w_arr = np.zeros((3, 2, E, 384), dtype=np.float32)
        b_arr = np.zeros((1, 2, 384), dtype=np.float32)
        for p, wm in enumerate((wq, wk, wv)):
            for hh in range(HPC):
                rows = slice((h0 + hh) * D, (h0 + hh + 1) * D)
                wr = wm[rows].real.T.astype(np.float32)   # [E, D]
                wi = wm[rows].imag.T.astype(np.float32)
                cs = slice(hh * 128, hh * 128 + 128)
                w_arr[p, 0, :, cs.start:cs.stop:2] = wr
                w_arr[p, 0, :, cs.start + 1:cs.stop:2] = wi
                w_arr[p, 1, :, cs.start:cs.stop:2] = -wi
                w_arr[p, 1, :, cs.start + 1:cs.stop:2] = wr
        for p, bm in enumerate((bq, bk)):
            for hh in range(HPC):
                rows = slice((h0 + hh) * D, (h0 + hh + 1) * D)
                br = bm[rows].real.astype(np.float32)
                bi = bm[rows].imag.astype(np.float32)
                cs = slice(hh * 128, hh * 128 + 128)
                b_arr[0, p, cs.start:cs.stop:2] = br
                b_arr[0, p, cs.start + 1:cs.stop:2] = bi
        in_maps.append({
            "xt_r": np.ascontiguousarray(x_real[b].T).astype(bf16),
            "xt_i": np.ascontiguousarray(x_imag[b].T).astype(bf16),
            "w": w_arr.astype(bf16),
            "bias": b_arr,
        })
    return in_maps


def _run(x_real, x_imag, w_qkv, b_qkv, qn_w, kn_w, trace=False):
    import time as _t
    if "nc" not in _prog_cache:
        t0 = _t.time()
        _prog_cache["nc"] = build_program()
        print(f"[kernel] program built in {_t.time() - t0:.1f}s", flush=True)
    nc = _prog_cache["nc"]
    t0 = _t.time()
    in_maps = _host_prep(x_real, x_imag, w_qkv, b_qkv, qn_w, kn_w)
    print(f"[kernel] host prep {_t.time() - t0:.1f}s", flush=True)
    t0 = _t.time()
    try:
        res = run_bass_kernel_spmd(nc, in_maps, list(range(8)), trace=trace)
    except Exception as e:
        if not trace:
            raise
        print(f"[kernel] trace run failed ({e!r}); retrying without trace",
              flush=True)
        res = run_bass_kernel_spmd(nc, in_maps, list(range(8)), trace=False)
    print(f"[kernel] device run {_t.time() - t0:.1f}s", flush=True)

    full = np.zeros((B, N, E), dtype=np.complex64)
    bv = b_qkv[2 * E:3 * E]                       # v bias, applied on host
    for c in range(8):
        b = c // 2
        h0 = HPC * (c % 2)
        outT = res.results[c]["outT"].astype(np.float32)  # [3,2,128,1024]
        zac = res.results[c]["zac"].astype(np.float32)    # [3,2,128,2,1024]
        for hh in range(HPC):
            for qc in range(QC):
                z = zac[hh, qc].sum(axis=(0, 1))          # [1024]
                o = outT[hh, qc] / z[None, :]             # [128, 1024]
                oc = (o[0::2] + 1j * o[1::2]).T           # [1024, 64]
                h = h0 + hh
                full[b, qc * 1024:(qc + 1) * 1024,
                     h * D:(h + 1) * D] = oc + bv[h * D:(h + 1) * D]
    return full, res


def kernel(x_real, x_imag, w_qkv, b_qkv, qn_w, kn_w):
    full, _ = _run(x_real, x_imag, w_qkv, b_qkv, qn_w, kn_w, trace=False)
    return full


def kernel_profiled(x_real, x_imag, w_qkv, b_qkv, qn_w, kn_w):
    return _run(x_real, x_imag, w_qkv, b_qkv, qn_w, kn_w, trace=True)
